# revision 6
# baseline (speedup 1.0000x reference)
"""Trainium2 Bass kernel for nn_ClusteringLayer (gnn_message_passing).

Computation (see reference):
  cov    = segment_sum(x_cov.reshape(N, F*F), mask_labels, C)   # [C, F*F]
  x_corr = correlation readout of cov                           # [C, F]
  mask   = hard gumbel softmax of MLP(BN(x_corr))               # [C, 64]

Sharding: nodes (N) split evenly over 8 NeuronCores. Each core computes a
partial segment-sum via one-hot matmuls accumulated in PSUM, then the
partials are AllReduce'd (per column-group, overlapped with compute) and
the tiny correlation/MLP/gumbel epilogue is computed redundantly on every
core. Core 0's outputs are returned.

Self-contained: hardcodes all shapes; only needs numpy + concourse.
"""

import numpy as np

import concourse.bass as bass
import concourse.mybir as mybir
import concourse.bacc as bacc
import concourse.tile as tile
from concourse import bass_utils
from concourse.masks import make_identity

# ---- problem constants ----
N = 100000
C = 128              # input clusters (segments)
F = 116
FF = F * F           # 13456
NK = 64              # output clusters
EPS_BN = 1e-5
EPS_CORR = 1e-12

NCORES = 8
P = 128
NSHARD = N // NCORES                 # 12500 nodes per core
NCHUNK = (NSHARD + P - 1) // P       # 98 chunks of 128 nodes
NPAD = NCHUNK * P                    # 12544
LAST_ROWS = NSHARD - (NCHUNK - 1) * P  # 84 valid rows in the last chunk

GROUP_W = 4096                       # col-group width = 8 PSUM banks x 512
GROUPS = []
_c0 = 0
while _c0 < FF:
    GROUPS.append((_c0, min(GROUP_W, FF - _c0)))
    _c0 += GROUP_W

f32 = mybir.dt.float32
i32 = mybir.dt.int32
AX = mybir.AxisListType
ALU = mybir.AluOpType
ACT = mybir.ActivationFunctionType


def build():
    nc = bacc.Bacc("TRN2", target_bir_lowering=False, debug=False,
                   num_devices=NCORES)

    # -------- I/O --------
    xc = nc.dram_tensor("xc", [NSHARD, FF], f32, kind="ExternalInput")
    labels_t = nc.dram_tensor("labels_t", [P, NCHUNK], f32, kind="ExternalInput")
    w1t = nc.dram_tensor("w1t", [F + 1, 256], f32, kind="ExternalInput")
    g1be1 = nc.dram_tensor("g1be1", [F, 2], f32, kind="ExternalInput")
    w2s = nc.dram_tensor("w2s", [P, 128], f32, kind="ExternalInput")
    b2row = nc.dram_tensor("b2row", [1, NK], f32, kind="ExternalInput")
    g2be2 = nc.dram_tensor("g2be2", [P, 4], f32, kind="ExternalInput")
    gu = nc.dram_tensor("gu", [C, NK], f32, kind="ExternalInput")

    cov_out = nc.dram_tensor("cov_out", [C, FF], f32, kind="ExternalOutput")
    xcorr_out = nc.dram_tensor("xcorr_out", [C, F], f32, kind="ExternalOutput")
    mask_out = nc.dram_tensor("mask_out", [C, NK], f32, kind="ExternalOutput")

    with tile.TileContext(nc) as tc:
        with (
            tc.tile_pool(name="const", bufs=1) as const,
            tc.tile_pool(name="io", bufs=3) as io,
            tc.tile_pool(name="cpy", bufs=3) as cpy,
            tc.tile_pool(name="epi", bufs=1) as epi,
            tc.tile_pool(name="psum", bufs=1, space="PSUM") as psum,
            tc.tile_pool(name="dram", bufs=1, space="DRAM") as dram,
        ):
            # ---- prologue: one-hot matrix for all node chunks ----
            labels_sb = const.tile([P, NCHUNK], f32)
            nc.sync.dma_start(out=labels_sb[:], in_=labels_t[:])

            iota_i = const.tile([P, C], i32)
            nc.gpsimd.iota(iota_i[:], pattern=[[1, C]], base=0, channel_multiplier=0)
            iota_f = const.tile([P, C], f32)
            nc.vector.tensor_copy(out=iota_f[:], in_=iota_i[:])

            onehot = const.tile([P, NCHUNK * C], f32)
            for k in range(NCHUNK):
                nc.vector.tensor_scalar(
                    out=onehot[:, k * C:(k + 1) * C],
                    in0=iota_f[:],
                    scalar1=labels_sb[:, k:k + 1],
                    scalar2=None,
                    op0=ALU.is_equal,
                )

            # small epilogue inputs
            identity = const.tile([P, P], f32)
            make_identity(nc, identity[:])
            w1sb = const.tile([F + 1, 256], f32)
            nc.sync.dma_start(out=w1sb[:], in_=w1t[:])
            g1be1_sb = const.tile([F, 2], f32)
            nc.sync.dma_start(out=g1be1_sb[:], in_=g1be1[:])
            w2_sb = const.tile([P, 128], f32)
            nc.sync.dma_start(out=w2_sb[:], in_=w2s[:])
            b2_sb = const.tile([1, NK], f32)
            nc.sync.dma_start(out=b2_sb[:], in_=b2row[:])
            g2be2_sb = const.tile([P, 4], f32)
            nc.sync.dma_start(out=g2be2_sb[:], in_=g2be2[:])
            gu_sb = const.tile([C, NK], f32)
            nc.sync.dma_start(out=gu_sb[:], in_=gu[:])
            ones1 = const.tile([1, P], f32)
            nc.vector.memset(ones1[:], 1.0)
            epsbn = const.tile([P, 1], f32)
            nc.vector.memset(epsbn[:], EPS_BN)

            # DRAM bounce buffers for the collectives (per column group)
            ar_in = []
            ar_out = []
            for gi, (g0, gw) in enumerate(GROUPS):
                t_in = dram.tile([C, gw], f32, name=f"ar_in{gi}")
                t_out = dram.tile([C, gw], f32, addr_space="Shared",
                                  name=f"ar_out{gi}")
                ar_in.append(t_in)
                ar_out.append(t_out)

            # ---- main loop: partial segment-sum via one-hot matmuls ----
            for gi, (g0, gw) in enumerate(GROUPS):
                nsub = (gw + 511) // 512
                accs = [
                    psum.tile([P, 512], f32, tag=f"acc{s}", name=f"acc{gi}_{s}")
                    for s in range(nsub)
                ]
                for k in range(NCHUNK):
                    rows = LAST_ROWS if k == NCHUNK - 1 else P
                    xtile = io.tile([P, gw], f32, tag="xt", name=f"xt{gi}_{k}")
                    nc.sync.dma_start(
                        out=xtile[:rows, :],
                        in_=xc[k * P:k * P + rows, g0:g0 + gw],
                    )
                    lhsT = onehot[:, k * C:(k + 1) * C]
                    for s in range(nsub):
                        w = min(512, gw - s * 512)
                        nc.tensor.matmul(
                            out=accs[s][:, :w],
                            lhsT=lhsT,
                            rhs=xtile[:, s * 512:s * 512 + w],
                            start=(k == 0),
                            stop=(k == NCHUNK - 1),
                        )
                # drain group: PSUM -> SBUF -> DRAM bounce, then AllReduce
                for s in range(nsub):
                    w = min(512, gw - s * 512)
                    bank_sb = cpy.tile([P, 512], f32, tag="bank",
                                       name=f"bank{gi}_{s}")
                    nc.scalar.copy(out=bank_sb[:, :w], in_=accs[s][:, :w])
                    nc.sync.dma_start(
                        out=ar_in[gi][:, s * 512:s * 512 + w],
                        in_=bank_sb[:, :w],
                    )
                nc.gpsimd.collective_compute(
                    "AllReduce",
                    ALU.add,
                    replica_groups=[list(range(NCORES))],
                    ins=[ar_in[gi][:]],
                    outs=[ar_out[gi][:]],
                )
                # reduced cov slice straight to the output (DRAM -> DRAM)
                nc.sync.dma_start(out=cov_out[:, g0:g0 + gw], in_=ar_out[gi][:])

            # ---- epilogue (identical on every core) ----
            cov_sb = epi.tile([C, FF], f32)
            for gi, (g0, gw) in enumerate(GROUPS):
                nc.sync.dma_start(out=cov_sb[:, g0:g0 + gw], in_=ar_out[gi][:])

            # d = sqrt(clip(diag(cov), 0))
            d_sb = epi.tile([C, F], f32)
            nc.vector.tensor_scalar(
                out=d_sb[:], in0=cov_sb[:, 0:FF:F + 1],
                scalar1=0.0, scalar2=None, op0=ALU.max,
            )
            nc.scalar.sqrt(out=d_sb[:], in_=d_sb[:])

            # x_corr = mean_j cov[:, i, j] / (d_i * d_j + eps)
            xcs = epi.tile([C, F], f32)
            BLK = 29
            for b in range(F // BLK):
                i0 = b * BLK
                den = epi.tile([C, BLK * F], f32, tag="den", name=f"den{b}")
                den3 = den[:].rearrange("p (i j) -> p i j", i=BLK)
                d_i = d_sb[:, i0:i0 + BLK].to_broadcast([C, BLK, F])
                d_j = d_sb[:, 0:F].unsqueeze(1).broadcast_to([C, BLK, F])
                nc.vector.tensor_tensor(out=den3, in0=d_i, in1=d_j, op=ALU.mult)
                nc.vector.tensor_scalar(
                    out=den[:], in0=den[:], scalar1=EPS_CORR, scalar2=None,
                    op0=ALU.add,
                )
                nc.vector.reciprocal(out=den[:], in_=den[:])
                nc.vector.tensor_tensor(
                    out=den[:], in0=den[:],
                    in1=cov_sb[:, i0 * F:(i0 + BLK) * F], op=ALU.mult,
                )
                nc.vector.reduce_sum(out=xcs[:, i0:i0 + BLK], in_=den3, axis=AX.X)
            nc.scalar.mul(out=xcs[:], in_=xcs[:], mul=1.0 / F)
            nc.sync.dma_start(out=xcorr_out[:], in_=xcs[:])

            # ---- BN1 (over clusters) in transposed layout ----
            xt_ps = psum.tile([F, P], f32, tag="acc0")
            nc.tensor.transpose(out=xt_ps[:], in_=xcs[:], identity=identity[:])
            # rows 0..115 = bn1(x_corr).T; row 116 stays 1.0 (bias row for the
            # fused matmul). Partition-base-0 ops only: fill all rows with 1.0
            # first, then overwrite the first 116.
            xt = epi.tile([P, P], f32)
            nc.vector.memset(xt[:], 1.0)
            nc.vector.tensor_copy(out=xt[:F, :], in_=xt_ps[:])

            mu1 = epi.tile([F, 1], f32)
            nc.vector.reduce_sum(out=mu1[:], in_=xt[:F, :], axis=AX.X)
            nc.scalar.mul(out=mu1[:], in_=mu1[:], mul=1.0 / C)
            nc.vector.tensor_scalar(
                out=xt[:F, :], in0=xt[:F, :], scalar1=mu1[:, 0:1], scalar2=None,
                op0=ALU.subtract,
            )
            sq1 = epi.tile([F, P], f32)
            v1 = epi.tile([F, 1], f32)
            nc.scalar.activation(out=sq1[:], in_=xt[:F, :], func=ACT.Square,
                                 accum_out=v1[:, 0:1])
            nc.scalar.activation(out=v1[:], in_=v1[:], func=ACT.Sqrt,
                                 scale=1.0 / C, bias=epsbn[:F, 0:1])
            rstd1 = epi.tile([F, 1], f32)
            nc.vector.reciprocal(out=rstd1[:], in_=v1[:])
            s1 = epi.tile([F, 1], f32)
            nc.vector.tensor_tensor(out=s1[:], in0=g1be1_sb[:, 0:1],
                                    in1=rstd1[:], op=ALU.mult)
            nc.vector.tensor_scalar(
                out=xt[:F, :], in0=xt[:F, :],
                scalar1=s1[:, 0:1], scalar2=g1be1_sb[:, 1:2],
                op0=ALU.mult, op1=ALU.add,
            )

            # h = relu(bn1(x) @ w1.T + b1): [C, 256]
            h_ps = psum.tile([C, 256], f32, tag="acc1")
            nc.tensor.matmul(out=h_ps[:], lhsT=xt[:F + 1, :], rhs=w1sb[:],
                             start=True, stop=True)
            h_sb = epi.tile([C, 256], f32)
            nc.scalar.activation(out=h_sb[:], in_=h_ps[:], func=ACT.Relu)

            # ---- BN2 in transposed layout ([128, 2x128]) ----
            ht = epi.tile([P, 256], f32)
            for blk in range(2):
                ht_ps = psum.tile([P, P], f32, tag=f"acc{2 + blk}",
                                  name=f"ht_ps{blk}")
                nc.tensor.transpose(out=ht_ps[:],
                                    in_=h_sb[:, blk * P:(blk + 1) * P],
                                    identity=identity[:])
                nc.vector.tensor_copy(out=ht[:, blk * P:(blk + 1) * P],
                                      in_=ht_ps[:])

            mu2 = epi.tile([P, 2], f32)
            nc.vector.reduce_sum(
                out=mu2[:], in_=ht[:].rearrange("p (b c) -> p b c", b=2),
                axis=AX.X,
            )
            nc.scalar.mul(out=mu2[:], in_=mu2[:], mul=1.0 / C)
            v2 = epi.tile([P, 2], f32)
            sq2 = epi.tile([P, P], f32)
            for blk in range(2):
                nc.vector.tensor_scalar(
                    out=ht[:, blk * P:(blk + 1) * P],
                    in0=ht[:, blk * P:(blk + 1) * P],
                    scalar1=mu2[:, blk:blk + 1], scalar2=None,
                    op0=ALU.subtract,
                )
                nc.scalar.activation(out=sq2[:], in_=ht[:, blk * P:(blk + 1) * P],
                                     func=ACT.Square,
                                     accum_out=v2[:, blk:blk + 1])
            nc.scalar.activation(out=v2[:], in_=v2[:], func=ACT.Sqrt,
                                 scale=1.0 / C, bias=epsbn[:, 0:1])
            rstd2 = epi.tile([P, 2], f32)
            nc.vector.reciprocal(out=rstd2[:], in_=v2[:])
            s2 = epi.tile([P, 2], f32)
            nc.vector.tensor_tensor(out=s2[:], in0=g2be2_sb[:, 0:2],
                                    in1=rstd2[:], op=ALU.mult)
            for blk in range(2):
                nc.vector.tensor_scalar(
                    out=ht[:, blk * P:(blk + 1) * P],
                    in0=ht[:, blk * P:(blk + 1) * P],
                    scalar1=s2[:, blk:blk + 1],
                    scalar2=g2be2_sb[:, 2 + blk:3 + blk],
                    op0=ALU.mult, op1=ALU.add,
                )

            # logits = bn2(h) @ w2.T + b2: [C, 64]
            lg_ps = psum.tile([C, NK], f32, tag="acc4")
            nc.tensor.matmul(out=lg_ps[:], lhsT=ht[:, 0:P], rhs=w2_sb[:, 0:NK],
                             start=True, stop=False)
            nc.tensor.matmul(out=lg_ps[:], lhsT=ht[:, P:2 * P],
                             rhs=w2_sb[:, NK:2 * NK], start=False, stop=False)
            nc.tensor.matmul(out=lg_ps[:], lhsT=ones1[:], rhs=b2_sb[:],
                             start=False, stop=True)

            # gumbels = -log(-log(clip(u)))
            ug = epi.tile([C, NK], f32)
            nc.vector.tensor_scalar(
                out=ug[:], in0=gu_sb[:], scalar1=1e-6, scalar2=1.0 - 1e-6,
                op0=ALU.max, op1=ALU.min,
            )
            nc.scalar.activation(out=ug[:], in_=ug[:], func=ACT.Ln)
            nc.scalar.activation(out=ug[:], in_=ug[:], func=ACT.Ln, scale=-1.0)
            # z = logits + gumbels = logits - ug
            z = epi.tile([C, NK], f32)
            nc.vector.tensor_tensor(out=z[:], in0=lg_ps[:], in1=ug[:],
                                    op=ALU.subtract)

            # y_soft = softmax(z, axis=1)
            mx = epi.tile([C, 1], f32)
            nc.vector.reduce_max(out=mx[:], in_=z[:], axis=AX.X)
            nmx = epi.tile([C, 1], f32)
            nc.scalar.mul(out=nmx[:], in_=mx[:], mul=-1.0)
            es = epi.tile([C, NK], f32)
            se = epi.tile([C, 1], f32)
            nc.scalar.activation(out=es[:], in_=z[:], func=ACT.Exp,
                                 bias=nmx[:, 0:1], accum_out=se[:, 0:1])
            rs = epi.tile([C, 1], f32)
            nc.vector.reciprocal(out=rs[:], in_=se[:])
            ys = epi.tile([C, NK], f32)
            nc.vector.tensor_scalar(out=ys[:], in0=es[:], scalar1=rs[:, 0:1],
                                    scalar2=None, op0=ALU.mult)

            # y_hard = onehot(argmax(y_soft)); out = y_hard - y_soft + y_soft
            mx2 = epi.tile([C, 1], f32)
            nc.vector.reduce_max(out=mx2[:], in_=ys[:], axis=AX.X)
            mo = epi.tile([C, NK], f32)
            nc.vector.tensor_scalar(out=mo[:], in0=ys[:], scalar1=mx2[:, 0:1],
                                    scalar2=None, op0=ALU.is_equal)
            nc.vector.tensor_tensor(out=mo[:], in0=mo[:], in1=ys[:],
                                    op=ALU.subtract)
            nc.vector.tensor_tensor(out=mo[:], in0=mo[:], in1=ys[:],
                                    op=ALU.add)
            nc.sync.dma_start(out=mask_out[:], in_=mo[:])

    nc.compile()
    return nc


_NC_CACHE = None


def _get_nc():
    global _NC_CACHE
    if _NC_CACHE is None:
        _NC_CACHE = build()
    return _NC_CACHE


def make_in_maps(x_cov, mask_labels, gumbel_u, w1, b1, g1, be1, w2, b2, g2, be2):
    flat = np.ascontiguousarray(x_cov, dtype=np.float32).reshape(N, FF)
    labels = np.asarray(mask_labels).astype(np.int64)

    # shared (replicated) small inputs
    w1t = np.concatenate(
        [np.asarray(w1, np.float32).T, np.asarray(b1, np.float32)[None, :]], axis=0
    )  # [117, 256]
    g1be1 = np.stack(
        [np.asarray(g1, np.float32), np.asarray(be1, np.float32)], axis=1
    )  # [116, 2]
    w2t = np.asarray(w2, np.float32).T            # [256, 64], rows = o
    w2s = np.concatenate([w2t[:P], w2t[P:]], axis=1)  # [128, 128]
    b2row = np.asarray(b2, np.float32)[None, :]   # [1, 64]
    g2r = np.asarray(g2, np.float32).reshape(2, P).T   # [128, 2]
    be2r = np.asarray(be2, np.float32).reshape(2, P).T  # [128, 2]
    g2be2 = np.concatenate([g2r, be2r], axis=1)   # [128, 4]
    gu = np.ascontiguousarray(gumbel_u, dtype=np.float32)

    shared = {
        "w1t": np.ascontiguousarray(w1t),
        "g1be1": np.ascontiguousarray(g1be1),
        "w2s": np.ascontiguousarray(w2s),
        "b2row": np.ascontiguousarray(b2row),
        "g2be2": np.ascontiguousarray(g2be2),
        "gu": gu,
    }

    in_maps = []
    for c in range(NCORES):
        lab = np.full(NPAD, -1.0, np.float32)
        lab[:NSHARD] = labels[c * NSHARD:(c + 1) * NSHARD]
        lab_t = np.ascontiguousarray(lab.reshape(NCHUNK, P).T)  # [128, 98]
        in_maps.append({
            "xc": flat[c * NSHARD:(c + 1) * NSHARD],
            "labels_t": lab_t,
            **shared,
        })
    return in_maps


def kernel(x_cov, mask_labels, gumbel_u, w1, b1, g1, be1, w2, b2, g2, be2):
    in_maps = make_in_maps(x_cov, mask_labels, gumbel_u, w1, b1, g1, be1,
                           w2, b2, g2, be2)
    nc = _get_nc()
    res = bass_utils.run_bass_kernel_spmd(nc, in_maps,
                                          core_ids=list(range(NCORES)))
    r0 = res.results[0]
    cov = r0["cov_out"].reshape(C, F, F)
    return cov, r0["xcorr_out"], r0["mask_out"]


# revision 11
# speedup vs baseline: 47.6564x; 47.6564x over previous
"""Trainium2 Bass kernel for nn_ClusteringLayer (gnn_message_passing).

Computation (see reference):
  cov    = segment_sum(x_cov.reshape(N, F*F), mask_labels, C)   # [C, F*F]
  x_corr = correlation readout of cov                           # [C, F]
  mask   = hard gumbel softmax of MLP(BN(x_corr))               # [C, 64]

Sharding: nodes (N) split evenly over 8 NeuronCores. Each core computes a
partial segment-sum via one-hot matmuls accumulated in PSUM, then the
partials are AllReduce'd (per column-group, overlapped with compute) and
the tiny correlation/MLP/gumbel epilogue is computed redundantly on every
core. Core 0's outputs are returned.

Self-contained: hardcodes all shapes; only needs numpy + concourse.
"""

import numpy as np

import concourse.bass as bass
import concourse.mybir as mybir
import concourse.bacc as bacc
import concourse.tile as tile
from concourse import bass_utils
from concourse.masks import make_identity

# ---- problem constants ----
N = 100000
C = 128              # input clusters (segments)
F = 116
FF = F * F           # 13456
NK = 64              # output clusters
EPS_BN = 1e-5
EPS_CORR = 1e-12

NCORES = 8
P = 128
NSHARD = N // NCORES                 # 12500 nodes per core
NCHUNK = (NSHARD + P - 1) // P       # 98 chunks of 128 nodes
NPAD = NCHUNK * P                    # 12544
LAST_ROWS = NSHARD - (NCHUNK - 1) * P  # 84 valid rows in the last chunk

GROUP_W = 4096                       # col-group width = 8 PSUM banks x 512
GROUPS = []
_c0 = 0
while _c0 < FF:
    GROUPS.append((_c0, min(GROUP_W, FF - _c0)))
    _c0 += GROUP_W

f32 = mybir.dt.float32
i32 = mybir.dt.int32
AX = mybir.AxisListType
ALU = mybir.AluOpType
ACT = mybir.ActivationFunctionType


def emit_mlp_gumbel(nc, tc, epi, psum, identity, w1sb, g1be1_sb, w2_sb, b2_sb,
                    g2be2_sb, gu_sb, ones1, epsbn, xcs, mo, stop_at="full"):
    """BN1 -> lin -> relu -> BN2 -> lin -> hard gumbel softmax.

    Reads x_corr from `xcs` [C, F]; writes the straight-through mask to `mo`
    [C, NK]. All tiles tiny; runs once per core after the AllReduce.
    """
    # ---- BN1 (over clusters) in transposed layout ----
    xt_ps = psum.tile([F, P], f32, tag="acc0")
    nc.tensor.transpose(out=xt_ps[:], in_=xcs[:], identity=identity[:])
    # rows 0..115 = bn1(x_corr).T; row 116 stays 1.0 (bias row for the
    # fused matmul). Partition-base-0 ops only: fill all rows with 1.0
    # first, then overwrite the first 116.
    xt = epi.tile([P, P], f32)
    nc.vector.memset(xt[:], 1.0)
    nc.vector.tensor_copy(out=xt[:F, :], in_=xt_ps[:])

    mu1 = epi.tile([F, 1], f32)
    nc.vector.reduce_sum(out=mu1[:], in_=xt[:F, :], axis=AX.X)
    nc.scalar.mul(out=mu1[:], in_=mu1[:], mul=1.0 / C)
    nc.vector.tensor_scalar(
        out=xt[:F, :], in0=xt[:F, :], scalar1=mu1[:, 0:1], scalar2=None,
        op0=ALU.subtract,
    )
    sq1 = epi.tile([F, P], f32)
    v1 = epi.tile([F, 1], f32)
    nc.scalar.activation(out=sq1[:], in_=xt[:F, :], func=ACT.Square,
                         accum_out=v1[:, 0:1])
    nc.scalar.activation(out=v1[:], in_=v1[:], func=ACT.Sqrt,
                         scale=1.0 / C, bias=epsbn[:F, 0:1])
    rstd1 = epi.tile([F, 1], f32)
    nc.vector.reciprocal(out=rstd1[:], in_=v1[:])
    s1 = epi.tile([F, 1], f32)
    nc.vector.tensor_tensor(out=s1[:], in0=g1be1_sb[:, 0:1],
                            in1=rstd1[:], op=ALU.mult)
    nc.vector.tensor_scalar(
        out=xt[:F, :], in0=xt[:F, :],
        scalar1=s1[:, 0:1], scalar2=g1be1_sb[:, 1:2],
        op0=ALU.mult, op1=ALU.add,
    )

    # h = relu(bn1(x) @ w1.T + b1): [C, 256]
    h_ps = psum.tile([C, 256], f32, tag="acc1")
    nc.tensor.matmul(out=h_ps[:], lhsT=xt[:F + 1, :], rhs=w1sb[:],
                     start=True, stop=True)
    h_sb = epi.tile([C, 256], f32)
    nc.scalar.activation(out=h_sb[:], in_=h_ps[:], func=ACT.Relu)
    if stop_at == "bn1":
        nc.vector.tensor_copy(out=mo[:], in_=h_sb[:, 0:NK])
        return

    # ---- BN2 in transposed layout ([128, 2x128]) ----
    ht = epi.tile([P, 256], f32)
    for blk in range(2):
        ht_ps = psum.tile([P, P], f32, tag=f"acc{2 + blk}",
                          name=f"ht_ps{blk}")
        nc.tensor.transpose(out=ht_ps[:],
                            in_=h_sb[:, blk * P:(blk + 1) * P],
                            identity=identity[:])
        nc.vector.tensor_copy(out=ht[:, blk * P:(blk + 1) * P],
                              in_=ht_ps[:])

    mu2 = epi.tile([P, 2], f32)
    nc.vector.reduce_sum(
        out=mu2[:], in_=ht[:].rearrange("p (b c) -> p b c", b=2),
        axis=AX.X,
    )
    nc.scalar.mul(out=mu2[:], in_=mu2[:], mul=1.0 / C)
    v2 = epi.tile([P, 2], f32)
    sq2 = epi.tile([P, P], f32)
    for blk in range(2):
        nc.vector.tensor_scalar(
            out=ht[:, blk * P:(blk + 1) * P],
            in0=ht[:, blk * P:(blk + 1) * P],
            scalar1=mu2[:, blk:blk + 1], scalar2=None,
            op0=ALU.subtract,
        )
        nc.scalar.activation(out=sq2[:], in_=ht[:, blk * P:(blk + 1) * P],
                             func=ACT.Square,
                             accum_out=v2[:, blk:blk + 1])
    nc.scalar.activation(out=v2[:], in_=v2[:], func=ACT.Sqrt,
                         scale=1.0 / C, bias=epsbn[:, 0:1])
    rstd2 = epi.tile([P, 2], f32)
    nc.vector.reciprocal(out=rstd2[:], in_=v2[:])
    s2 = epi.tile([P, 2], f32)
    nc.vector.tensor_tensor(out=s2[:], in0=g2be2_sb[:, 0:2],
                            in1=rstd2[:], op=ALU.mult)
    for blk in range(2):
        nc.vector.tensor_scalar(
            out=ht[:, blk * P:(blk + 1) * P],
            in0=ht[:, blk * P:(blk + 1) * P],
            scalar1=s2[:, blk:blk + 1],
            scalar2=g2be2_sb[:, 2 + blk:3 + blk],
            op0=ALU.mult, op1=ALU.add,
        )

    # logits = bn2(h) @ w2.T + b2: [C, 64]
    lg_ps = psum.tile([C, NK], f32, tag="acc4")
    nc.tensor.matmul(out=lg_ps[:], lhsT=ht[:, 0:P], rhs=w2_sb[:, 0:NK],
                     start=True, stop=False)
    nc.tensor.matmul(out=lg_ps[:], lhsT=ht[:, P:2 * P],
                     rhs=w2_sb[:, NK:2 * NK], start=False, stop=False)
    nc.tensor.matmul(out=lg_ps[:], lhsT=ones1[:], rhs=b2_sb[:],
                     start=False, stop=True)

    # gumbels = -log(-log(clip(u)))
    ug = epi.tile([C, NK], f32)
    nc.vector.tensor_scalar(
        out=ug[:], in0=gu_sb[:], scalar1=1e-6, scalar2=1.0 - 1e-6,
        op0=ALU.max, op1=ALU.min,
    )
    nc.scalar.activation(out=ug[:], in_=ug[:], func=ACT.Ln)
    nc.scalar.activation(out=ug[:], in_=ug[:], func=ACT.Ln, scale=-1.0)
    # z = logits + gumbels = logits - ug
    z = epi.tile([C, NK], f32)
    nc.vector.tensor_tensor(out=z[:], in0=lg_ps[:], in1=ug[:],
                            op=ALU.subtract)

    # y_soft = softmax(z, axis=1)
    mx = epi.tile([C, 1], f32)
    nc.vector.reduce_max(out=mx[:], in_=z[:], axis=AX.X)
    nmx = epi.tile([C, 1], f32)
    nc.scalar.mul(out=nmx[:], in_=mx[:], mul=-1.0)
    es = epi.tile([C, NK], f32)
    se = epi.tile([C, 1], f32)
    nc.scalar.activation(out=es[:], in_=z[:], func=ACT.Exp,
                         bias=nmx[:, 0:1], accum_out=se[:, 0:1])
    rs = epi.tile([C, 1], f32)
    nc.vector.reciprocal(out=rs[:], in_=se[:])
    ys = epi.tile([C, NK], f32)
    nc.vector.tensor_scalar(out=ys[:], in0=es[:], scalar1=rs[:, 0:1],
                            scalar2=None, op0=ALU.mult)

    # y_hard = onehot(argmax(y_soft)); out = y_hard - y_soft + y_soft
    mx2 = epi.tile([C, 1], f32)
    nc.vector.reduce_max(out=mx2[:], in_=ys[:], axis=AX.X)
    nc.vector.tensor_scalar(out=mo[:], in0=ys[:], scalar1=mx2[:, 0:1],
                            scalar2=None, op0=ALU.is_equal)
    nc.vector.tensor_tensor(out=mo[:], in0=mo[:], in1=ys[:],
                            op=ALU.subtract)
    nc.vector.tensor_tensor(out=mo[:], in0=mo[:], in1=ys[:],
                            op=ALU.add)


def build(reps=1):
    """Build the full SPMD program. reps>1 replicates the whole body for
    differential wall-clock timing (t(reps=a) - t(reps=b))/(a-b)."""
    nc = bacc.Bacc("TRN2", target_bir_lowering=False, debug=False,
                   num_devices=NCORES)

    # -------- I/O --------
    xc = nc.dram_tensor("xc", [NSHARD, FF], f32, kind="ExternalInput")
    labels_t = nc.dram_tensor("labels_t", [P, NCHUNK], f32, kind="ExternalInput")
    w1t = nc.dram_tensor("w1t", [F + 1, 256], f32, kind="ExternalInput")
    g1be1 = nc.dram_tensor("g1be1", [F, 2], f32, kind="ExternalInput")
    w2s = nc.dram_tensor("w2s", [P, 128], f32, kind="ExternalInput")
    b2row = nc.dram_tensor("b2row", [1, NK], f32, kind="ExternalInput")
    g2be2 = nc.dram_tensor("g2be2", [P, 4], f32, kind="ExternalInput")
    gu = nc.dram_tensor("gu", [C, NK], f32, kind="ExternalInput")

    cov_out = nc.dram_tensor("cov_out", [C, FF], f32, kind="ExternalOutput")
    xcorr_out = nc.dram_tensor("xcorr_out", [C, F], f32, kind="ExternalOutput")
    mask_out = nc.dram_tensor("mask_out", [C, NK], f32, kind="ExternalOutput")

    with tile.TileContext(nc) as tc:
        with (
            tc.tile_pool(name="const", bufs=1) as const,
            tc.tile_pool(name="io", bufs=3) as io,
            tc.tile_pool(name="cpy", bufs=3) as cpy,
            tc.tile_pool(name="epi", bufs=1) as epi,
            tc.tile_pool(name="psum", bufs=1, space="PSUM") as psum,
            tc.tile_pool(name="dram", bufs=1, space="DRAM") as dram,
        ):
            # ---- prologue: one-hot matrix for all node chunks ----
            labels_sb = const.tile([P, NCHUNK], f32)
            nc.sync.dma_start(out=labels_sb[:], in_=labels_t[:])

            iota_i = const.tile([P, C], i32)
            nc.gpsimd.iota(iota_i[:], pattern=[[1, C]], base=0, channel_multiplier=0)
            iota_f = const.tile([P, C], f32)
            nc.vector.tensor_copy(out=iota_f[:], in_=iota_i[:])

            onehot = const.tile([P, NCHUNK * C], f32)
            for k in range(NCHUNK):
                nc.vector.tensor_scalar(
                    out=onehot[:, k * C:(k + 1) * C],
                    in0=iota_f[:],
                    scalar1=labels_sb[:, k:k + 1],
                    scalar2=None,
                    op0=ALU.is_equal,
                )

            # small epilogue inputs
            identity = const.tile([P, P], f32)
            make_identity(nc, identity[:])
            w1sb = const.tile([F + 1, 256], f32)
            nc.sync.dma_start(out=w1sb[:], in_=w1t[:])
            g1be1_sb = const.tile([F, 2], f32)
            nc.sync.dma_start(out=g1be1_sb[:], in_=g1be1[:])
            w2_sb = const.tile([P, 128], f32)
            nc.sync.dma_start(out=w2_sb[:], in_=w2s[:])
            b2_sb = const.tile([1, NK], f32)
            nc.sync.dma_start(out=b2_sb[:], in_=b2row[:])
            g2be2_sb = const.tile([P, 4], f32)
            nc.sync.dma_start(out=g2be2_sb[:], in_=g2be2[:])
            gu_sb = const.tile([C, NK], f32)
            nc.sync.dma_start(out=gu_sb[:], in_=gu[:])
            ones1 = const.tile([1, P], f32)
            nc.vector.memset(ones1[:], 1.0)
            epsbn = const.tile([P, 1], f32)
            nc.vector.memset(epsbn[:], EPS_BN)

            for rep in range(reps):
                _emit_rep(nc, tc, rep, xc, cov_out, xcorr_out, mask_out,
                          onehot, identity, w1sb, g1be1_sb, w2_sb, b2_sb,
                          g2be2_sb, gu_sb, ones1, epsbn,
                          io, cpy, epi, psum, dram)

    nc.compile()
    return nc


def _emit_rep(nc, tc, rep, xc, cov_out, xcorr_out, mask_out, onehot, identity,
              w1sb, g1be1_sb, w2_sb, b2_sb, g2be2_sb, gu_sb, ones1, epsbn,
              io, cpy, epi, psum, dram):
            # DRAM bounce buffers for the collectives (per column group)
            ar_in = []
            ar_out = []
            for gi, (g0, gw) in enumerate(GROUPS):
                t_in = dram.tile([C, gw], f32, name=f"ar_in{gi}_{rep}",
                                 tag=f"ar_in{gi}")
                t_out = dram.tile([C, gw], f32, addr_space="Shared",
                                  name=f"ar_out{gi}_{rep}", tag=f"ar_out{gi}")
                ar_in.append(t_in)
                ar_out.append(t_out)

            # ---- main loop: partial segment-sum via one-hot matmuls ----
            for gi, (g0, gw) in enumerate(GROUPS):
                nsub = (gw + 511) // 512
                accs = [
                    psum.tile([P, 512], f32, tag=f"acc{s}", name=f"acc{gi}_{s}")
                    for s in range(nsub)
                ]
                for k in range(NCHUNK):
                    rows = LAST_ROWS if k == NCHUNK - 1 else P
                    xtile = io.tile([P, gw], f32, tag="xt", name=f"xt{gi}_{k}")
                    nc.sync.dma_start(
                        out=xtile[:rows, :],
                        in_=xc[k * P:k * P + rows, g0:g0 + gw],
                    )
                    lhsT = onehot[:, k * C:(k + 1) * C]
                    for s in range(nsub):
                        w = min(512, gw - s * 512)
                        nc.tensor.matmul(
                            out=accs[s][:, :w],
                            lhsT=lhsT,
                            rhs=xtile[:, s * 512:s * 512 + w],
                            start=(k == 0),
                            stop=(k == NCHUNK - 1),
                        )
                # drain group: PSUM -> SBUF -> DRAM bounce, then AllReduce
                for s in range(nsub):
                    w = min(512, gw - s * 512)
                    bank_sb = cpy.tile([P, 512], f32, tag="bank",
                                       name=f"bank{gi}_{s}")
                    nc.scalar.copy(out=bank_sb[:, :w], in_=accs[s][:, :w])
                    nc.sync.dma_start(
                        out=ar_in[gi][:, s * 512:s * 512 + w],
                        in_=bank_sb[:, :w],
                    )
                nc.gpsimd.collective_compute(
                    "AllReduce",
                    ALU.add,
                    replica_groups=[list(range(NCORES))],
                    ins=[ar_in[gi][:]],
                    outs=[ar_out[gi][:]],
                )
                # reduced cov slice straight to the output (DRAM -> DRAM)
                nc.sync.dma_start(out=cov_out[:, g0:g0 + gw], in_=ar_out[gi][:])

            # ---- epilogue (identical on every core) ----
            cov_sb = epi.tile([C, FF], f32)
            for gi, (g0, gw) in enumerate(GROUPS):
                nc.sync.dma_start(out=cov_sb[:, g0:g0 + gw], in_=ar_out[gi][:])

            # d = sqrt(clip(diag(cov), 0))
            d_sb = epi.tile([C, F], f32)
            nc.vector.tensor_scalar(
                out=d_sb[:], in0=cov_sb[:, 0:FF:F + 1],
                scalar1=0.0, scalar2=None, op0=ALU.max,
            )
            nc.scalar.sqrt(out=d_sb[:], in_=d_sb[:])

            # x_corr = mean_j cov[:, i, j] / (d_i * d_j + eps)
            xcs = epi.tile([C, F], f32)
            BLK = 29
            for b in range(F // BLK):
                i0 = b * BLK
                den = epi.tile([C, BLK * F], f32, tag="den", name=f"den{b}")
                den3 = den[:].rearrange("p (i j) -> p i j", i=BLK)
                d_i = d_sb[:, i0:i0 + BLK].to_broadcast([C, BLK, F])
                d_j = d_sb[:, 0:F].unsqueeze(1).broadcast_to([C, BLK, F])
                nc.vector.tensor_tensor(out=den3, in0=d_i, in1=d_j, op=ALU.mult)
                nc.vector.tensor_scalar(
                    out=den[:], in0=den[:], scalar1=EPS_CORR, scalar2=None,
                    op0=ALU.add,
                )
                nc.vector.reciprocal(out=den[:], in_=den[:])
                nc.vector.tensor_tensor(
                    out=den[:], in0=den[:],
                    in1=cov_sb[:, i0 * F:(i0 + BLK) * F], op=ALU.mult,
                )
                nc.vector.reduce_sum(out=xcs[:, i0:i0 + BLK], in_=den3, axis=AX.X)
            nc.scalar.mul(out=xcs[:], in_=xcs[:], mul=1.0 / F)
            nc.sync.dma_start(out=xcorr_out[:], in_=xcs[:])

            mo = epi.tile([C, NK], f32)
            emit_mlp_gumbel(nc, tc, epi, psum, identity, w1sb, g1be1_sb,
                            w2_sb, b2_sb, g2be2_sb, gu_sb, ones1, epsbn,
                            xcs, mo)
            nc.sync.dma_start(out=mask_out[:], in_=mo[:])


_NC_CACHE = None


def _get_nc():
    global _NC_CACHE
    if _NC_CACHE is None:
        _NC_CACHE = build()
    return _NC_CACHE


def make_in_maps(x_cov, mask_labels, gumbel_u, w1, b1, g1, be1, w2, b2, g2, be2):
    flat = np.ascontiguousarray(x_cov, dtype=np.float32).reshape(N, FF)
    labels = np.asarray(mask_labels).astype(np.int64)

    # shared (replicated) small inputs
    w1t = np.concatenate(
        [np.asarray(w1, np.float32).T, np.asarray(b1, np.float32)[None, :]], axis=0
    )  # [117, 256]
    g1be1 = np.stack(
        [np.asarray(g1, np.float32), np.asarray(be1, np.float32)], axis=1
    )  # [116, 2]
    w2t = np.asarray(w2, np.float32).T            # [256, 64], rows = o
    w2s = np.concatenate([w2t[:P], w2t[P:]], axis=1)  # [128, 128]
    b2row = np.asarray(b2, np.float32)[None, :]   # [1, 64]
    g2r = np.asarray(g2, np.float32).reshape(2, P).T   # [128, 2]
    be2r = np.asarray(be2, np.float32).reshape(2, P).T  # [128, 2]
    g2be2 = np.concatenate([g2r, be2r], axis=1)   # [128, 4]
    gu = np.ascontiguousarray(gumbel_u, dtype=np.float32)

    shared = {
        "w1t": np.ascontiguousarray(w1t),
        "g1be1": np.ascontiguousarray(g1be1),
        "w2s": np.ascontiguousarray(w2s),
        "b2row": np.ascontiguousarray(b2row),
        "g2be2": np.ascontiguousarray(g2be2),
        "gu": gu,
    }

    in_maps = []
    for c in range(NCORES):
        lab = np.full(NPAD, -1.0, np.float32)
        lab[:NSHARD] = labels[c * NSHARD:(c + 1) * NSHARD]
        lab_t = np.ascontiguousarray(lab.reshape(NCHUNK, P).T)  # [128, 98]
        in_maps.append({
            "xc": flat[c * NSHARD:(c + 1) * NSHARD],
            "labels_t": lab_t,
            **shared,
        })
    return in_maps


def kernel(x_cov, mask_labels, gumbel_u, w1, b1, g1, be1, w2, b2, g2, be2):
    in_maps = make_in_maps(x_cov, mask_labels, gumbel_u, w1, b1, g1, be1,
                           w2, b2, g2, be2)
    nc = _get_nc()
    res = bass_utils.run_bass_kernel_spmd(nc, in_maps,
                                          core_ids=list(range(NCORES)))
    r0 = res.results[0]
    cov = r0["cov_out"].reshape(C, F, F)
    return cov, r0["xcorr_out"], r0["mask_out"]


# revision 14
# speedup vs baseline: 102.0241x; 2.1408x over previous
"""Trainium2 Bass kernel for nn_ClusteringLayer (gnn_message_passing).

Computation (see reference):
  cov    = segment_sum(x_cov.reshape(N, F*F), mask_labels, C)   # [C, F*F]
  x_corr = correlation readout of cov                           # [C, F]
  mask   = hard gumbel softmax of MLP(BN(x_corr))               # [C, 64]

Sharding: nodes (N) split evenly over 8 NeuronCores. Each core computes a
partial segment-sum via one-hot matmuls accumulated in PSUM, then the
partials are AllReduce'd (per column-group, overlapped with compute) and
the tiny correlation/MLP/gumbel epilogue is computed redundantly on every
core. Core 0's outputs are returned.

Self-contained: hardcodes all shapes; only needs numpy + concourse.
"""

import numpy as np

import concourse.bass as bass
import concourse.mybir as mybir
import concourse.bacc as bacc
import concourse.tile as tile
from concourse import bass_utils
from concourse.masks import make_identity

# ---- problem constants ----
N = 100000
C = 128              # input clusters (segments)
F = 116
FF = F * F           # 13456
NK = 64              # output clusters
EPS_BN = 1e-5
EPS_CORR = 1e-12

NCORES = 8
P = 128
NSHARD = N // NCORES                 # 12500 nodes per core
NCHUNK = (NSHARD + P - 1) // P       # 98 chunks of 128 nodes
NPAD = NCHUNK * P                    # 12544
LAST_ROWS = NSHARD - (NCHUNK - 1) * P  # 84 valid rows in the last chunk

GROUP_W = 4096                       # col-group width = 8 PSUM banks x 512
GROUPS = []
_c0 = 0
while _c0 < FF:
    GROUPS.append((_c0, min(GROUP_W, FF - _c0)))
    _c0 += GROUP_W

f32 = mybir.dt.float32
i32 = mybir.dt.int32
AX = mybir.AxisListType
ALU = mybir.AluOpType
ACT = mybir.ActivationFunctionType


def emit_mlp_gumbel(nc, tc, epi, psum, identity, w1sb, g1be1_sb, w2_sb, b2_sb,
                    g2be2_sb, gu_sb, ones1, epsbn, xcs, mo, stop_at="full"):
    """BN1 -> lin -> relu -> BN2 -> lin -> hard gumbel softmax.

    Reads x_corr from `xcs` [C, F]; writes the straight-through mask to `mo`
    [C, NK]. All tiles tiny; runs once per core after the AllReduce.
    """
    # ---- BN1 (over clusters) in transposed layout ----
    xt_ps = psum.tile([F, P], f32, tag="acc0")
    nc.tensor.transpose(out=xt_ps[:], in_=xcs[:], identity=identity[:])
    # rows 0..115 = bn1(x_corr).T; row 116 stays 1.0 (bias row for the
    # fused matmul). Partition-base-0 ops only: fill all rows with 1.0
    # first, then overwrite the first 116.
    xt = epi.tile([P, P], f32)
    nc.vector.memset(xt[:], 1.0)
    nc.vector.tensor_copy(out=xt[:F, :], in_=xt_ps[:])

    mu1 = epi.tile([F, 1], f32)
    nc.vector.reduce_sum(out=mu1[:], in_=xt[:F, :], axis=AX.X)
    nc.scalar.mul(out=mu1[:], in_=mu1[:], mul=1.0 / C)
    nc.vector.tensor_scalar(
        out=xt[:F, :], in0=xt[:F, :], scalar1=mu1[:, 0:1], scalar2=None,
        op0=ALU.subtract,
    )
    sq1 = epi.tile([F, P], f32)
    v1 = epi.tile([F, 1], f32)
    nc.scalar.activation(out=sq1[:], in_=xt[:F, :], func=ACT.Square,
                         accum_out=v1[:, 0:1])
    nc.scalar.activation(out=v1[:], in_=v1[:], func=ACT.Sqrt,
                         scale=1.0 / C, bias=epsbn[:F, 0:1])
    rstd1 = epi.tile([F, 1], f32)
    nc.vector.reciprocal(out=rstd1[:], in_=v1[:])
    s1 = epi.tile([F, 1], f32)
    nc.vector.tensor_tensor(out=s1[:], in0=g1be1_sb[:, 0:1],
                            in1=rstd1[:], op=ALU.mult)
    nc.vector.tensor_scalar(
        out=xt[:F, :], in0=xt[:F, :],
        scalar1=s1[:, 0:1], scalar2=g1be1_sb[:, 1:2],
        op0=ALU.mult, op1=ALU.add,
    )

    # h = relu(bn1(x) @ w1.T + b1): [C, 256]
    h_ps = psum.tile([C, 256], f32, tag="acc1")
    nc.tensor.matmul(out=h_ps[:], lhsT=xt[:F + 1, :], rhs=w1sb[:],
                     start=True, stop=True)
    h_sb = epi.tile([C, 256], f32)
    nc.scalar.activation(out=h_sb[:], in_=h_ps[:], func=ACT.Relu)
    if stop_at == "bn1":
        nc.vector.tensor_copy(out=mo[:], in_=h_sb[:, 0:NK])
        return

    # ---- BN2 in transposed layout ([128, 2x128]) ----
    ht = epi.tile([P, 256], f32)
    for blk in range(2):
        ht_ps = psum.tile([P, P], f32, tag=f"acc{2 + blk}",
                          name=f"ht_ps{blk}")
        nc.tensor.transpose(out=ht_ps[:],
                            in_=h_sb[:, blk * P:(blk + 1) * P],
                            identity=identity[:])
        nc.vector.tensor_copy(out=ht[:, blk * P:(blk + 1) * P],
                              in_=ht_ps[:])

    mu2 = epi.tile([P, 2], f32)
    nc.vector.reduce_sum(
        out=mu2[:], in_=ht[:].rearrange("p (b c) -> p b c", b=2),
        axis=AX.X,
    )
    nc.scalar.mul(out=mu2[:], in_=mu2[:], mul=1.0 / C)
    v2 = epi.tile([P, 2], f32)
    sq2 = epi.tile([P, P], f32)
    for blk in range(2):
        nc.vector.tensor_scalar(
            out=ht[:, blk * P:(blk + 1) * P],
            in0=ht[:, blk * P:(blk + 1) * P],
            scalar1=mu2[:, blk:blk + 1], scalar2=None,
            op0=ALU.subtract,
        )
        nc.scalar.activation(out=sq2[:], in_=ht[:, blk * P:(blk + 1) * P],
                             func=ACT.Square,
                             accum_out=v2[:, blk:blk + 1])
    nc.scalar.activation(out=v2[:], in_=v2[:], func=ACT.Sqrt,
                         scale=1.0 / C, bias=epsbn[:, 0:1])
    rstd2 = epi.tile([P, 2], f32)
    nc.vector.reciprocal(out=rstd2[:], in_=v2[:])
    s2 = epi.tile([P, 2], f32)
    nc.vector.tensor_tensor(out=s2[:], in0=g2be2_sb[:, 0:2],
                            in1=rstd2[:], op=ALU.mult)
    for blk in range(2):
        nc.vector.tensor_scalar(
            out=ht[:, blk * P:(blk + 1) * P],
            in0=ht[:, blk * P:(blk + 1) * P],
            scalar1=s2[:, blk:blk + 1],
            scalar2=g2be2_sb[:, 2 + blk:3 + blk],
            op0=ALU.mult, op1=ALU.add,
        )

    # logits = bn2(h) @ w2.T + b2: [C, 64]
    lg_ps = psum.tile([C, NK], f32, tag="acc4")
    nc.tensor.matmul(out=lg_ps[:], lhsT=ht[:, 0:P], rhs=w2_sb[:, 0:NK],
                     start=True, stop=False)
    nc.tensor.matmul(out=lg_ps[:], lhsT=ht[:, P:2 * P],
                     rhs=w2_sb[:, NK:2 * NK], start=False, stop=False)
    nc.tensor.matmul(out=lg_ps[:], lhsT=ones1[:], rhs=b2_sb[:],
                     start=False, stop=True)

    # gumbels = -log(-log(clip(u)))
    ug = epi.tile([C, NK], f32)
    nc.vector.tensor_scalar(
        out=ug[:], in0=gu_sb[:], scalar1=1e-6, scalar2=1.0 - 1e-6,
        op0=ALU.max, op1=ALU.min,
    )
    nc.scalar.activation(out=ug[:], in_=ug[:], func=ACT.Ln)
    nc.scalar.activation(out=ug[:], in_=ug[:], func=ACT.Ln, scale=-1.0)
    # z = logits + gumbels = logits - ug
    z = epi.tile([C, NK], f32)
    nc.vector.tensor_tensor(out=z[:], in0=lg_ps[:], in1=ug[:],
                            op=ALU.subtract)

    # y_soft = softmax(z, axis=1)
    mx = epi.tile([C, 1], f32)
    nc.vector.reduce_max(out=mx[:], in_=z[:], axis=AX.X)
    nmx = epi.tile([C, 1], f32)
    nc.scalar.mul(out=nmx[:], in_=mx[:], mul=-1.0)
    es = epi.tile([C, NK], f32)
    se = epi.tile([C, 1], f32)
    nc.scalar.activation(out=es[:], in_=z[:], func=ACT.Exp,
                         bias=nmx[:, 0:1], accum_out=se[:, 0:1])
    rs = epi.tile([C, 1], f32)
    nc.vector.reciprocal(out=rs[:], in_=se[:])
    ys = epi.tile([C, NK], f32)
    nc.vector.tensor_scalar(out=ys[:], in0=es[:], scalar1=rs[:, 0:1],
                            scalar2=None, op0=ALU.mult)

    # y_hard = onehot(argmax(y_soft)); out = y_hard - y_soft + y_soft
    mx2 = epi.tile([C, 1], f32)
    nc.vector.reduce_max(out=mx2[:], in_=ys[:], axis=AX.X)
    nc.vector.tensor_scalar(out=mo[:], in0=ys[:], scalar1=mx2[:, 0:1],
                            scalar2=None, op0=ALU.is_equal)
    nc.vector.tensor_tensor(out=mo[:], in0=mo[:], in1=ys[:],
                            op=ALU.subtract)
    nc.vector.tensor_tensor(out=mo[:], in0=mo[:], in1=ys[:],
                            op=ALU.add)


def build(reps=1, mode="full"):
    """Build the full SPMD program. reps>1 replicates the whole body for
    differential wall-clock timing (t(reps=a) - t(reps=b))/(a-b).
    mode: 'full' | 'mm' (main loop only) | 'mm_ar' (no epilogue)."""
    nc = bacc.Bacc("TRN2", target_bir_lowering=False, debug=False,
                   num_devices=NCORES)

    # -------- I/O --------
    xc = nc.dram_tensor("xc", [NSHARD, FF], f32, kind="ExternalInput")
    labels_t = nc.dram_tensor("labels_t", [P, NCHUNK], f32, kind="ExternalInput")
    w1t = nc.dram_tensor("w1t", [F + 1, 256], f32, kind="ExternalInput")
    g1be1 = nc.dram_tensor("g1be1", [F, 2], f32, kind="ExternalInput")
    w2s = nc.dram_tensor("w2s", [P, 128], f32, kind="ExternalInput")
    b2row = nc.dram_tensor("b2row", [1, NK], f32, kind="ExternalInput")
    g2be2 = nc.dram_tensor("g2be2", [P, 4], f32, kind="ExternalInput")
    gu = nc.dram_tensor("gu", [C, NK], f32, kind="ExternalInput")

    cov_out = nc.dram_tensor("cov_out", [C, FF], f32, kind="ExternalOutput")
    xcorr_out = nc.dram_tensor("xcorr_out", [C, F], f32, kind="ExternalOutput")
    mask_out = nc.dram_tensor("mask_out", [C, NK], f32, kind="ExternalOutput")

    with tile.TileContext(nc) as tc:
        with (
            tc.tile_pool(name="const", bufs=1) as const,
            tc.tile_pool(name="io", bufs=3) as io,
            tc.tile_pool(name="cpy", bufs=3) as cpy,
            tc.tile_pool(name="epi", bufs=1) as epi,
            tc.tile_pool(name="psum", bufs=1, space="PSUM") as psum,
            tc.tile_pool(name="dram", bufs=1, space="DRAM") as dram,
        ):
            # ---- prologue: one-hot matrix for all node chunks ----
            labels_sb = const.tile([P, NCHUNK], f32)
            nc.sync.dma_start(out=labels_sb[:], in_=labels_t[:])

            iota_i = const.tile([P, C], i32)
            nc.gpsimd.iota(iota_i[:], pattern=[[1, C]], base=0, channel_multiplier=0)
            iota_f = const.tile([P, C], f32)
            nc.vector.tensor_copy(out=iota_f[:], in_=iota_i[:])

            onehot = const.tile([P, NCHUNK * C], f32)
            for k in range(NCHUNK):
                nc.vector.tensor_scalar(
                    out=onehot[:, k * C:(k + 1) * C],
                    in0=iota_f[:],
                    scalar1=labels_sb[:, k:k + 1],
                    scalar2=None,
                    op0=ALU.is_equal,
                )

            # small epilogue inputs
            identity = const.tile([P, P], f32)
            make_identity(nc, identity[:])
            w1sb = const.tile([F + 1, 256], f32)
            nc.sync.dma_start(out=w1sb[:], in_=w1t[:])
            g1be1_sb = const.tile([F, 2], f32)
            nc.sync.dma_start(out=g1be1_sb[:], in_=g1be1[:])
            w2_sb = const.tile([P, 128], f32)
            nc.sync.dma_start(out=w2_sb[:], in_=w2s[:])
            b2_sb = const.tile([1, NK], f32)
            nc.sync.dma_start(out=b2_sb[:], in_=b2row[:])
            g2be2_sb = const.tile([P, 4], f32)
            nc.sync.dma_start(out=g2be2_sb[:], in_=g2be2[:])
            gu_sb = const.tile([C, NK], f32)
            nc.sync.dma_start(out=gu_sb[:], in_=gu[:])
            ones1 = const.tile([1, P], f32)
            nc.vector.memset(ones1[:], 1.0)
            epsbn = const.tile([P, 1], f32)
            nc.vector.memset(epsbn[:], EPS_BN)

            for rep in range(reps):
                _emit_rep(nc, tc, rep, xc, cov_out, xcorr_out, mask_out,
                          onehot, identity, w1sb, g1be1_sb, w2_sb, b2_sb,
                          g2be2_sb, gu_sb, ones1, epsbn,
                          io, cpy, epi, psum, dram, mode)
            if mode != "full":
                # keep xcorr/mask outputs written so the NEFF interface and
                # DCE behave identically across modes
                stub = epi.tile([C, F], f32, name="stub")
                nc.vector.memset(stub[:], 0.0)
                nc.sync.dma_start(out=xcorr_out[:], in_=stub[:])
                nc.sync.dma_start(out=mask_out[:], in_=stub[:, 0:NK])

    nc.compile()
    return nc


def _emit_rep(nc, tc, rep, xc, cov_out, xcorr_out, mask_out, onehot, identity,
              w1sb, g1be1_sb, w2_sb, b2_sb, g2be2_sb, gu_sb, ones1, epsbn,
              io, cpy, epi, psum, dram, mode="full"):
            # DRAM bounce buffers for the collectives (per column group)
            ar_in = []
            ar_out = []
            for gi, (g0, gw) in enumerate(GROUPS):
                t_in = dram.tile([C, gw], f32, name=f"ar_in{gi}_{rep}",
                                 tag=f"ar_in{gi}")
                t_out = dram.tile([C, gw], f32, addr_space="Shared",
                                  name=f"ar_out{gi}_{rep}", tag=f"ar_out{gi}")
                ar_in.append(t_in)
                ar_out.append(t_out)

            # ---- main loop: partial segment-sum via one-hot matmuls ----
            for gi, (g0, gw) in enumerate(GROUPS):
                nsub = (gw + 511) // 512
                accs = [
                    psum.tile([P, 512], f32, tag=f"acc{s}", name=f"acc{gi}_{s}")
                    for s in range(nsub)
                ]
                for k in range(NCHUNK):
                    rows = LAST_ROWS if k == NCHUNK - 1 else P
                    xtile = io.tile([P, gw], f32, tag="xt", name=f"xt{gi}_{k}")
                    nc.sync.dma_start(
                        out=xtile[:rows, :],
                        in_=xc[k * P:k * P + rows, g0:g0 + gw],
                    )
                    lhsT = onehot[:, k * C:(k + 1) * C]
                    for s in range(nsub):
                        w = min(512, gw - s * 512)
                        nc.tensor.matmul(
                            out=accs[s][:, :w],
                            lhsT=lhsT,
                            rhs=xtile[:, s * 512:s * 512 + w],
                            start=(k == 0),
                            stop=(k == NCHUNK - 1),
                        )
                # drain group: PSUM -> SBUF -> DRAM bounce, then AllReduce
                for s in range(nsub):
                    w = min(512, gw - s * 512)
                    bank_sb = cpy.tile([P, 512], f32, tag="bank",
                                       name=f"bank{gi}_{s}")
                    nc.scalar.copy(out=bank_sb[:, :w], in_=accs[s][:, :w])
                    nc.sync.dma_start(
                        out=ar_in[gi][:, s * 512:s * 512 + w],
                        in_=bank_sb[:, :w],
                    )
                if mode == "mm":
                    nc.sync.dma_start(out=cov_out[:, g0:g0 + gw],
                                      in_=ar_in[gi][:])
                    continue
                nc.gpsimd.collective_compute(
                    "AllReduce",
                    ALU.add,
                    replica_groups=[list(range(NCORES))],
                    ins=[ar_in[gi][:]],
                    outs=[ar_out[gi][:]],
                )
                # reduced cov slice straight to the output (DRAM -> DRAM)
                nc.sync.dma_start(out=cov_out[:, g0:g0 + gw], in_=ar_out[gi][:])

            if mode in ("mm", "mm_ar"):
                return

            # ---- epilogue (identical on every core) ----
            cov_sb = epi.tile([C, FF], f32)
            for gi, (g0, gw) in enumerate(GROUPS):
                nc.sync.dma_start(out=cov_sb[:, g0:g0 + gw], in_=ar_out[gi][:])

            # d = sqrt(clip(diag(cov), 0))
            d_sb = epi.tile([C, F], f32)
            nc.vector.tensor_scalar(
                out=d_sb[:], in0=cov_sb[:, 0:FF:F + 1],
                scalar1=0.0, scalar2=None, op0=ALU.max,
            )
            nc.scalar.sqrt(out=d_sb[:], in_=d_sb[:])

            # x_corr = mean_j cov[:, i, j] / (d_i * d_j + eps)
            xcs = epi.tile([C, F], f32)
            BLK = 29
            for b in range(F // BLK):
                i0 = b * BLK
                den = epi.tile([C, BLK * F], f32, tag="den", name=f"den{b}")
                den3 = den[:].rearrange("p (i j) -> p i j", i=BLK)
                d_i = d_sb[:, i0:i0 + BLK].to_broadcast([C, BLK, F])
                d_j = d_sb[:, 0:F].unsqueeze(1).broadcast_to([C, BLK, F])
                nc.vector.tensor_tensor(out=den3, in0=d_i, in1=d_j, op=ALU.mult)
                nc.vector.tensor_scalar(
                    out=den[:], in0=den[:], scalar1=EPS_CORR, scalar2=None,
                    op0=ALU.add,
                )
                nc.vector.reciprocal(out=den[:], in_=den[:])
                nc.vector.tensor_tensor(
                    out=den[:], in0=den[:],
                    in1=cov_sb[:, i0 * F:(i0 + BLK) * F], op=ALU.mult,
                )
                nc.vector.reduce_sum(out=xcs[:, i0:i0 + BLK], in_=den3, axis=AX.X)
            nc.scalar.mul(out=xcs[:], in_=xcs[:], mul=1.0 / F)
            nc.sync.dma_start(out=xcorr_out[:], in_=xcs[:])

            mo = epi.tile([C, NK], f32)
            emit_mlp_gumbel(nc, tc, epi, psum, identity, w1sb, g1be1_sb,
                            w2_sb, b2_sb, g2be2_sb, gu_sb, ones1, epsbn,
                            xcs, mo)
            nc.sync.dma_start(out=mask_out[:], in_=mo[:])


_NC_CACHE = None


def _get_nc():
    global _NC_CACHE
    if _NC_CACHE is None:
        _NC_CACHE = build()
    return _NC_CACHE


def make_in_maps(x_cov, mask_labels, gumbel_u, w1, b1, g1, be1, w2, b2, g2, be2):
    flat = np.ascontiguousarray(x_cov, dtype=np.float32).reshape(N, FF)
    labels = np.asarray(mask_labels).astype(np.int64)

    # shared (replicated) small inputs
    w1t = np.concatenate(
        [np.asarray(w1, np.float32).T, np.asarray(b1, np.float32)[None, :]], axis=0
    )  # [117, 256]
    g1be1 = np.stack(
        [np.asarray(g1, np.float32), np.asarray(be1, np.float32)], axis=1
    )  # [116, 2]
    w2t = np.asarray(w2, np.float32).T            # [256, 64], rows = o
    w2s = np.concatenate([w2t[:P], w2t[P:]], axis=1)  # [128, 128]
    b2row = np.asarray(b2, np.float32)[None, :]   # [1, 64]
    g2r = np.asarray(g2, np.float32).reshape(2, P).T   # [128, 2]
    be2r = np.asarray(be2, np.float32).reshape(2, P).T  # [128, 2]
    g2be2 = np.concatenate([g2r, be2r], axis=1)   # [128, 4]
    gu = np.ascontiguousarray(gumbel_u, dtype=np.float32)

    shared = {
        "w1t": np.ascontiguousarray(w1t),
        "g1be1": np.ascontiguousarray(g1be1),
        "w2s": np.ascontiguousarray(w2s),
        "b2row": np.ascontiguousarray(b2row),
        "g2be2": np.ascontiguousarray(g2be2),
        "gu": gu,
    }

    in_maps = []
    for c in range(NCORES):
        lab = np.full(NPAD, -1.0, np.float32)
        lab[:NSHARD] = labels[c * NSHARD:(c + 1) * NSHARD]
        lab_t = np.ascontiguousarray(lab.reshape(NCHUNK, P).T)  # [128, 98]
        in_maps.append({
            "xc": flat[c * NSHARD:(c + 1) * NSHARD],
            "labels_t": lab_t,
            **shared,
        })
    return in_maps


def kernel(x_cov, mask_labels, gumbel_u, w1, b1, g1, be1, w2, b2, g2, be2):
    in_maps = make_in_maps(x_cov, mask_labels, gumbel_u, w1, b1, g1, be1,
                           w2, b2, g2, be2)
    nc = _get_nc()
    res = bass_utils.run_bass_kernel_spmd(nc, in_maps,
                                          core_ids=list(range(NCORES)))
    r0 = res.results[0]
    cov = r0["cov_out"].reshape(C, F, F)
    return cov, r0["xcorr_out"], r0["mask_out"]
